# revision 2
# baseline (speedup 1.0000x reference)
"""DeepseekV2-Lite MoE (group GEMM) on 8 TRN2 NeuronCores — v2.

Strategy (expert-parallel, host-routed, weight-stationary ragged GEMMs):
  - Host (numpy, fp32): gate softmax -> top-6, per-expert token lists.
    Experts sorted by load, rank-grouped into 8 slots x 8 cores so every
    core runs an identical program with slot widths w_j = group max
    (aligned 16). Dispatch gathers x columns into [H, S_pad] per core.
  - Device (per core, Bass/Tile, bf16 MMs, fp32 PSUM):
      * routed: per expert slot j, weight-stationary SwiGLU:
          lhsT = weight tiles [128H x 128F], rhs = x columns [128H, w_j]
          -> g,u in PSUM [128F, w_j]; silu(g)*u -> gu SBUF (bf16);
          down: lhsT = w_down tiles [128F x 128H], rhs = gu -> out
          [128H, w_j]. Token dim rides the matmul free axis: no 128-row
          padding, no PE transposes. Gate weights applied on host.
      * shared experts: same flipped structure on an F_SH/8 slice over
        4 chunks of 512 tokens.
  - Host: unweighted slot outputs * gate weight, scatter-add; sum shared
    partials across cores.
"""

import os
from contextlib import ExitStack
from dataclasses import dataclass, field

import ml_dtypes
import numpy as np

H = 2048
F = 1408
E = 64
TOP_K = 6
CAP = 512
SCALE = 1.0
F_SH = 2 * F
B, S = 1, 2048
T = B * S
N_CORES = 8
P = 128
KT = H // P          # 16 k-tiles over H
NF = F // P          # 11 f-tiles over F
FSH_LOC = F_SH // N_CORES   # 352
FSH_PAD = 384
NFS = FSH_PAD // P   # 3
SCH = 512            # shared-phase token chunk
NCH = T // SCH       # 4 chunks

BF16 = ml_dtypes.bfloat16

LAST_EXEC_NS = None
_LAST_RUN = None
_LAST_CFG = None


@dataclass
class MoECfg:
    widths: tuple = (240,) * 8   # per-slot token widths (aligned 16, <=512)

    @property
    def offs(self):
        o, acc = [], 0
        for w in self.widths:
            o.append(acc)
            acc += w
        return o

    @property
    def S_pad(self):
        return sum(self.widths)

    @property
    def W_MAX(self):
        return max(self.widths)

    @property
    def OUTW(self):
        return 16 * self.S_pad


def build_moe_program(cfg: MoECfg, repeat: int = 1):
    import contextlib
    import concourse.bass as bass  # noqa: F401
    import concourse.mybir as mybir
    from concourse import bacc
    import concourse.tile as tile

    bf = mybir.dt.bfloat16
    f32 = mybir.dt.float32
    SILU = mybir.ActivationFunctionType.Silu

    W_MAX, S_pad = cfg.W_MAX, cfg.S_pad

    nc = bacc.Bacc("TRN2", target_bir_lowering=False, debug=False)

    # ---- DRAM parameters -------------------------------------------------
    xdT = nc.dram_tensor("xdT", [KT, P, S_pad], bf, kind="ExternalInput").ap()
    wgu = nc.dram_tensor("wgu", [8, NF, KT, P, 2 * P], bf, kind="ExternalInput").ap()
    wd = nc.dram_tensor("wd", [8, NF, P, H], bf, kind="ExternalInput").ap()
    xT = nc.dram_tensor("xT", [KT, P, T], bf, kind="ExternalInput").ap()
    shgu = nc.dram_tensor("shgu", [NFS, KT, P, 2 * P], bf, kind="ExternalInput").ap()
    shd = nc.dram_tensor("shd", [NFS, P, H], bf, kind="ExternalInput").ap()

    d_out = nc.dram_tensor("d_out", [P, cfg.OUTW], bf, kind="ExternalOutput").ap()
    ysh = nc.dram_tensor("ysh", [P, 16 * T], bf, kind="ExternalOutput").ap()

    with tile.TileContext(nc) as tc:
        with ExitStack() as top:
            shw_pool = top.enter_context(tc.tile_pool(name="shw", bufs=1))
            xd_pool = top.enter_context(tc.tile_pool(name="xd", bufs=2))
            wgu_pool = top.enter_context(tc.tile_pool(name="wgu", bufs=4))
            wd_pool = top.enter_context(tc.tile_pool(name="wd", bufs=2))
            gu_pool = top.enter_context(tc.tile_pool(name="gu", bufs=1))
            out_pool = top.enter_context(tc.tile_pool(name="out", bufs=2))
            tmp_pool = top.enter_context(tc.tile_pool(name="tmp", bufs=2))
            xsh_pool = top.enter_context(tc.tile_pool(name="xsh", bufs=2))
            gush_pool = top.enter_context(tc.tile_pool(name="gush", bufs=2))
            psgu_pool = top.enter_context(tc.tile_pool(name="psgu", bufs=2, space="PSUM"))
            psd_pool = top.enter_context(tc.tile_pool(name="psd", bufs=4, space="PSUM"))

            # shared-expert weights: prefetch once at start (scalar queue)
            shgu_sb = shw_pool.tile([P, NFS, KT, 2 * P], bf)
            nc.scalar.dma_start(shgu_sb, shgu.rearrange("f k p c -> p f k c"))
            shd_sb = shw_pool.tile([P, NFS, H], bf)
            nc.scalar.dma_start(shd_sb, shd.rearrange("f p h -> p f h"))

            rep_ctx = tc.For_i(0, repeat, 1) if repeat > 1 else contextlib.nullcontext()
            with rep_ctx:
                # ================= routed experts (flipped) =================
                for j in range(8):
                    wj = cfg.widths[j]
                    off = cfg.offs[j]
                    xd = xd_pool.tile([P, KT, W_MAX], bf, tag="xd", name=f"xd{j}")
                    nc.sync.dma_start(
                        xd[:, :, :wj],
                        xdT[:, :, off : off + wj].rearrange("k p s -> p k s"),
                    )
                    gu = gu_pool.tile([P, NF, W_MAX], bf, tag="gu", name=f"gu{j}")
                    for f in range(NF):
                        wt = wgu_pool.tile([P, KT, 2 * P], bf, tag="wgu",
                                           name=f"wgu{j}_{f}")
                        nc.sync.dma_start(wt, wgu[j, f].rearrange("k p c -> p k c"))
                        psg = psgu_pool.tile([P, 512], f32, tag="psg", name=f"psg{j}_{f}")
                        psu = psgu_pool.tile([P, 512], f32, tag="psu", name=f"psu{j}_{f}")
                        for k in range(KT):
                            nc.tensor.matmul(psg[:, :wj], wt[:, k, 0:P], xd[:, k, :wj],
                                             start=(k == 0), stop=(k == KT - 1))
                            nc.tensor.matmul(psu[:, :wj], wt[:, k, P : 2 * P],
                                             xd[:, k, :wj],
                                             start=(k == 0), stop=(k == KT - 1))
                        sil = tmp_pool.tile([P, 512], f32, tag="sil", name=f"sil{j}_{f}")
                        nc.scalar.activation(sil[:, :wj], psg[:, :wj], SILU)
                        nc.vector.tensor_mul(gu[:, f, :wj], sil[:, :wj], psu[:, :wj])
                    outt = out_pool.tile([P, 16 * W_MAX], bf, tag="outt", name=f"out{j}")
                    for hc in range(4):  # H chunks of 512 cols
                        wdt = wd_pool.tile([P, NF, 512], bf, tag="wd",
                                           name=f"wd{j}_{hc}")
                        nc.scalar.dma_start(
                            wdt,
                            wd[j, :, :, hc * 512 : (hc + 1) * 512].rearrange(
                                "kf p h -> p kf h"),
                        )
                        for h in range(4):
                            ht = hc * 4 + h
                            psd = psd_pool.tile([P, 512], f32, tag="psd",
                                                name=f"psd{j}_{ht}")
                            for kf in range(NF):
                                nc.tensor.matmul(
                                    psd[:, :wj], wdt[:, kf, h * P : (h + 1) * P],
                                    gu[:, kf, :wj],
                                    start=(kf == 0), stop=(kf == NF - 1),
                                )
                            nc.scalar.copy(outt[:, ht * wj : (ht + 1) * wj],
                                           psd[:, :wj])
                    nc.gpsimd.dma_start(
                        d_out[:, 16 * off : 16 * off + 16 * wj], outt[:, : 16 * wj])

                # ================= shared experts (flipped, F_SH/8 slice) ====
                for c in range(NCH):
                    xs = xsh_pool.tile([P, KT, SCH], bf, tag="xs", name=f"xs{c}")
                    nc.sync.dma_start(
                        xs, xT[:, :, c * SCH : (c + 1) * SCH].rearrange("k p s -> p k s"))
                    gush = gush_pool.tile([P, NFS, SCH], bf, tag="gush", name=f"gush{c}")
                    for f in range(NFS):
                        psg = psgu_pool.tile([P, 512], f32, tag="psg", name=f"psgs{c}_{f}")
                        psu = psgu_pool.tile([P, 512], f32, tag="psu", name=f"psus{c}_{f}")
                        for k in range(KT):
                            nc.tensor.matmul(psg, shgu_sb[:, f, k, 0:P], xs[:, k, :],
                                             start=(k == 0), stop=(k == KT - 1))
                            nc.tensor.matmul(psu, shgu_sb[:, f, k, P : 2 * P],
                                             xs[:, k, :],
                                             start=(k == 0), stop=(k == KT - 1))
                        sil = tmp_pool.tile([P, 512], f32, tag="sil", name=f"sils{c}_{f}")
                        nc.scalar.activation(sil, psg, SILU)
                        nc.vector.tensor_mul(gush[:, f, :], sil, psu)
                    outsh = out_pool.tile([P, 16 * SCH], bf, tag="outsh", name=f"osh{c}")
                    for h in range(16):
                        psd = psd_pool.tile([P, 512], f32, tag="psd", name=f"psds{c}_{h}")
                        for kf in range(NFS):
                            nc.tensor.matmul(psd, shd_sb[:, kf, h * P : (h + 1) * P],
                                             gush[:, kf, :],
                                             start=(kf == 0), stop=(kf == NFS - 1))
                        nc.scalar.copy(outsh[:, h * SCH : (h + 1) * SCH], psd)
                    nc.gpsimd.dma_start(
                        ysh[:, c * 16 * SCH : (c + 1) * 16 * SCH], outsh)

    nc.compile()
    return nc


# ---------------------------------------------------------------------------
# Host-side routing / sharding / combine
# ---------------------------------------------------------------------------

def _route(x32, gate_w):
    logits = x32 @ gate_w.T.astype(np.float32)
    logits -= logits.max(-1, keepdims=True)
    np.exp(logits, out=logits)
    logits /= logits.sum(-1, keepdims=True)
    idx = np.argpartition(-logits, TOP_K - 1, axis=-1)[:, :TOP_K]
    w = np.take_along_axis(logits, idx, -1) * SCALE
    return idx.astype(np.int64), w.astype(np.float32)


def _align16(x):
    return min((int(x) + 15) // 16 * 16, 512)


def kernel(hidden_states, gate_w, w_gate, w_up, w_down, sh_gate, sh_up, sh_down):
    global LAST_EXEC_NS, _LAST_RUN, _LAST_CFG
    from concourse.bass_utils import run_bass_kernel_spmd

    x32 = np.ascontiguousarray(hidden_states, dtype=np.float32).reshape(T, H)

    # ---- host routing ----
    topk_idx, topk_w = _route(x32, np.asarray(gate_w, np.float32))
    eid = topk_idx.reshape(-1)
    order = np.argsort(eid, kind="stable")
    counts = np.bincount(eid, minlength=E)
    starts = np.concatenate([[0], np.cumsum(counts)[:-1]])
    wflat = topk_w.reshape(-1)

    # rank-group experts: slot j on all cores has width = max count in group j
    rank = np.argsort(-counts, kind="stable")          # experts by load desc
    groups = rank.reshape(8, N_CORES)                  # group j -> 8 experts
    gce = np.minimum(counts[groups], CAP)              # [8, 8] actual counts
    widths = tuple(_align16(gce[j].max()) for j in range(8))
    cfg = MoECfg(widths=widths)
    offs = cfg.offs

    # ---- per-core arrays ----
    x16T = np.ascontiguousarray(x32.T.astype(BF16))    # [H, T]
    wg16 = np.asarray(w_gate, np.float32).astype(BF16)
    wu16 = np.asarray(w_up, np.float32).astype(BF16)
    wdn16 = np.asarray(w_down, np.float32).astype(BF16)
    shg16 = np.asarray(sh_gate, np.float32).astype(BF16)
    shu16 = np.asarray(sh_up, np.float32).astype(BF16)
    shdn16 = np.asarray(sh_down, np.float32).astype(BF16)

    xT_arr = x16T.reshape(KT, P, T)

    def _gu_tiles(wmat):
        # [H, F] -> [NF, KT, P, P] lhsT tiles
        return wmat.reshape(KT, P, NF, P).transpose(2, 0, 1, 3)

    in_maps = []
    core_meta = []
    for c in range(N_CORES):
        es = groups[:, c]                              # expert id per slot
        tok = np.zeros(cfg.S_pad, np.int64)
        meta = []
        for j in range(8):
            e = int(es[j])
            ce = int(min(counts[e], CAP))
            rows = order[starts[e] : starts[e] + ce]
            tok[offs[j] : offs[j] + ce] = rows // TOP_K
            meta.append((j, ce, rows))
        core_meta.append(meta)

        xdT_arr = np.ascontiguousarray(x16T[:, tok]).reshape(KT, P, cfg.S_pad)

        wgu_arr = np.empty((8, NF, KT, P, 2 * P), BF16)
        wd_arr = np.empty((8, NF, P, H), BF16)
        for j in range(8):
            e = int(es[j])
            wgu_arr[j, :, :, :, 0:P] = _gu_tiles(wg16[e])
            wgu_arr[j, :, :, :, P : 2 * P] = _gu_tiles(wu16[e])
            wd_arr[j] = wdn16[e].reshape(NF, P, H)

        sl = slice(c * FSH_LOC, (c + 1) * FSH_LOC)
        sgp = np.zeros((H, FSH_PAD), BF16)
        sup = np.zeros((H, FSH_PAD), BF16)
        sgp[:, :FSH_LOC] = shg16[sl].T
        sup[:, :FSH_LOC] = shu16[sl].T
        shgu_arr = np.empty((NFS, KT, P, 2 * P), BF16)
        shgu_arr[..., 0:P] = sgp.reshape(KT, P, NFS, P).transpose(2, 0, 1, 3)
        shgu_arr[..., P : 2 * P] = sup.reshape(KT, P, NFS, P).transpose(2, 0, 1, 3)
        sdp = np.zeros((FSH_PAD, H), BF16)
        sdp[:FSH_LOC] = shdn16[:, sl].T
        shd_arr = sdp.reshape(NFS, P, H)

        in_maps.append({
            "xdT": xdT_arr,
            "wgu": np.ascontiguousarray(wgu_arr),
            "wd": np.ascontiguousarray(wd_arr),
            "xT": xT_arr,
            "shgu": np.ascontiguousarray(shgu_arr),
            "shd": np.ascontiguousarray(shd_arr),
        })

    # ---- build + run ----
    nc = build_moe_program(cfg)
    trace = os.environ.get("MOE_TRACE", "0") == "1"
    res = run_bass_kernel_spmd(
        nc, in_maps, core_ids=list(range(N_CORES)), trace=trace,
    )
    LAST_EXEC_NS = res.exec_time_ns
    _LAST_RUN = (nc, in_maps)
    _LAST_CFG = cfg

    # ---- combine ----
    out_exp = np.zeros((T * TOP_K, H), np.float32)
    ysh_sum = np.zeros((P, 16 * T), np.float32)
    for c in range(N_CORES):
        dcore = np.asarray(res.results[c]["d_out"], dtype=np.float32)
        for j, ce, rows in core_meta[c]:
            wj, off = cfg.widths[j], offs[j]
            blk = dcore[:, 16 * off : 16 * off + 16 * wj].reshape(P, 16, wj)
            d_e = blk.transpose(2, 1, 0).reshape(wj, H)[:ce]
            out_exp[rows] = d_e * wflat[rows][:, None]
        ysh_sum += np.asarray(res.results[c]["ysh"], dtype=np.float32)
    y = out_exp.reshape(T, TOP_K, H).sum(axis=1)
    y += ysh_sum.reshape(P, NCH, 16, SCH).transpose(1, 3, 2, 0).reshape(T, H)
    return y.reshape(B, S, H).astype(hidden_states.dtype)


# revision 8
# speedup vs baseline: 1.0719x; 1.0719x over previous
"""DeepseekV2-Lite MoE (group GEMM) on 8 TRN2 NeuronCores — v2.

Strategy (expert-parallel, host-routed, weight-stationary ragged GEMMs):
  - Host (numpy, fp32): gate softmax -> top-6, per-expert token lists.
    Experts sorted by load, rank-grouped into 8 slots x 8 cores so every
    core runs an identical program with slot widths w_j = group max
    (aligned 16). Dispatch gathers x columns into [H, S_pad] per core.
  - Device (per core, Bass/Tile, bf16 MMs, fp32 PSUM):
      * routed: per expert slot j, weight-stationary SwiGLU:
          lhsT = weight tiles [128H x 128F], rhs = x columns [128H, w_j]
          -> g,u in PSUM [128F, w_j]; silu(g)*u -> gu SBUF (bf16);
          down: lhsT = w_down tiles [128F x 128H], rhs = gu -> out
          [128H, w_j]. Token dim rides the matmul free axis: no 128-row
          padding, no PE transposes. Gate weights applied on host.
      * shared experts: same flipped structure on an F_SH/8 slice over
        4 chunks of 512 tokens.
  - Host: unweighted slot outputs * gate weight, scatter-add; sum shared
    partials across cores.
"""

import os
from contextlib import ExitStack
from dataclasses import dataclass, field

import ml_dtypes
import numpy as np

H = 2048
F = 1408
E = 64
TOP_K = 6
CAP = 512
SCALE = 1.0
F_SH = 2 * F
B, S = 1, 2048
T = B * S
N_CORES = 8
P = 128
KT = H // P          # 16 k-tiles over H
NF = F // P          # 11 f-tiles over F
FSH_LOC = F_SH // N_CORES   # 352
FSH_PAD = 384
NFS = FSH_PAD // P   # 3
SCH = 512            # shared-phase token chunk
NCH = T // SCH       # 4 chunks

BF16 = ml_dtypes.bfloat16

LAST_EXEC_NS = None
_LAST_RUN = None
_LAST_CFG = None


@dataclass
class MoECfg:
    widths: tuple = (240,) * 8   # per-slot token widths (aligned 16, <=512)

    @property
    def offs(self):
        o, acc = [], 0
        for w in self.widths:
            o.append(acc)
            acc += w
        return o

    @property
    def S_pad(self):
        return sum(self.widths)

    @property
    def W_MAX(self):
        return max(self.widths)

    @property
    def OUTW(self):
        return 16 * self.S_pad


def build_moe_program(cfg: MoECfg, repeat: int = 1):
    import contextlib
    import concourse.bass as bass  # noqa: F401
    import concourse.mybir as mybir
    from concourse import bacc
    import concourse.tile as tile

    bf = mybir.dt.bfloat16
    f32 = mybir.dt.float32
    SILU = mybir.ActivationFunctionType.Silu

    W_MAX, S_pad = cfg.W_MAX, cfg.S_pad

    nc = bacc.Bacc("TRN2", target_bir_lowering=False, debug=False)

    # ---- DRAM parameters (all pre-arranged host-side for fully-contiguous
    # per-partition DMA runs: no rearrange on the HBM side) -----------------
    xdT = nc.dram_tensor("xdT", [P, KT * S_pad], bf, kind="ExternalInput").ap()
    wgu = nc.dram_tensor("wgu", [8, NF, P, KT, 2 * P], bf, kind="ExternalInput").ap()
    wd = nc.dram_tensor("wd", [8, 4, P, NF, 512], bf, kind="ExternalInput").ap()
    xT = nc.dram_tensor("xT", [NCH, P, KT * SCH], bf, kind="ExternalInput").ap()
    shgu = nc.dram_tensor("shgu", [P, NFS, KT, 2 * P], bf, kind="ExternalInput").ap()
    shd = nc.dram_tensor("shd", [P, NFS, H], bf, kind="ExternalInput").ap()

    d_out = nc.dram_tensor("d_out", [P, cfg.OUTW], bf, kind="ExternalOutput").ap()
    ysh = nc.dram_tensor("ysh", [P, 16 * T], bf, kind="ExternalOutput").ap()

    with tile.TileContext(nc) as tc:
        with ExitStack() as top:
            shw_pool = top.enter_context(tc.tile_pool(name="shw", bufs=1))
            xd_pool = top.enter_context(tc.tile_pool(name="xd", bufs=2))
            wgu_pool = top.enter_context(tc.tile_pool(name="wgu", bufs=4))
            wd_pool = top.enter_context(tc.tile_pool(name="wd", bufs=2))
            gu_pool = top.enter_context(tc.tile_pool(name="gu", bufs=1))
            out_pool = top.enter_context(tc.tile_pool(name="out", bufs=2))
            tmp_pool = top.enter_context(tc.tile_pool(name="tmp", bufs=2))
            xsh_pool = top.enter_context(tc.tile_pool(name="xsh", bufs=2))
            gush_pool = top.enter_context(tc.tile_pool(name="gush", bufs=2))
            psgu_pool = top.enter_context(tc.tile_pool(name="psgu", bufs=2, space="PSUM"))
            psd_pool = top.enter_context(tc.tile_pool(name="psd", bufs=4, space="PSUM"))

            # shared-expert weights: prefetch once at start (scalar queue)
            shgu_sb = shw_pool.tile([P, NFS, KT, 2 * P], bf)
            nc.scalar.dma_start(shgu_sb, shgu)
            shd_sb = shw_pool.tile([P, NFS, H], bf)
            nc.scalar.dma_start(shd_sb, shd)

            rep_ctx = tc.For_i(0, repeat, 1) if repeat > 1 else contextlib.nullcontext()
            with rep_ctx:
                # ================= routed experts (flipped) =================
                for j in range(8):
                    wj = cfg.widths[j]
                    off = cfg.offs[j]
                    xd = xd_pool.tile([P, KT * W_MAX], bf, tag="xd", name=f"xd{j}")
                    nc.sync.dma_start(
                        xd[:, : KT * wj],
                        xdT[:, KT * off : KT * off + KT * wj],
                    )
                    gu = gu_pool.tile([P, NF, W_MAX], bf, tag="gu", name=f"gu{j}")
                    for f in range(NF):
                        wt = wgu_pool.tile([P, KT, 2 * P], bf, tag="wgu",
                                           name=f"wgu{j}_{f}")
                        nc.sync.dma_start(wt, wgu[j, f])
                        psg = psgu_pool.tile([P, 512], f32, tag="psg", name=f"psg{j}_{f}")
                        psu = psgu_pool.tile([P, 512], f32, tag="psu", name=f"psu{j}_{f}")
                        for k in range(KT):
                            nc.tensor.matmul(psg[:, :wj], wt[:, k, 0:P],
                                             xd[:, k * wj : (k + 1) * wj],
                                             start=(k == 0), stop=(k == KT - 1))
                            nc.tensor.matmul(psu[:, :wj], wt[:, k, P : 2 * P],
                                             xd[:, k * wj : (k + 1) * wj],
                                             start=(k == 0), stop=(k == KT - 1))
                        sil = tmp_pool.tile([P, 512], f32, tag="sil", name=f"sil{j}_{f}")
                        nc.scalar.activation(sil[:, :wj], psg[:, :wj], SILU)
                        nc.vector.tensor_mul(gu[:, f, :wj], sil[:, :wj], psu[:, :wj])
                    outt = out_pool.tile([P, 16 * W_MAX], bf, tag="outt", name=f"out{j}")
                    for hc in range(4):  # H chunks of 512 cols
                        wdt = wd_pool.tile([P, NF, 512], bf, tag="wd",
                                           name=f"wd{j}_{hc}")
                        nc.scalar.dma_start(wdt, wd[j, hc])
                        for h in range(4):
                            ht = hc * 4 + h
                            psd = psd_pool.tile([P, 512], f32, tag="psd",
                                                name=f"psd{j}_{ht}")
                            for kf in range(NF):
                                nc.tensor.matmul(
                                    psd[:, :wj], wdt[:, kf, h * P : (h + 1) * P],
                                    gu[:, kf, :wj],
                                    start=(kf == 0), stop=(kf == NF - 1),
                                )
                            nc.scalar.copy(outt[:, ht * wj : (ht + 1) * wj],
                                           psd[:, :wj])
                    nc.gpsimd.dma_start(
                        d_out[:, 16 * off : 16 * off + 16 * wj], outt[:, : 16 * wj])

                # ================= shared experts (flipped, F_SH/8 slice) ====
                for c in range(NCH):
                    xs = xsh_pool.tile([P, KT * SCH], bf, tag="xs", name=f"xs{c}")
                    nc.sync.dma_start(xs, xT[c])
                    gush = gush_pool.tile([P, NFS, SCH], bf, tag="gush", name=f"gush{c}")
                    for f in range(NFS):
                        psg = psgu_pool.tile([P, 512], f32, tag="psg", name=f"psgs{c}_{f}")
                        psu = psgu_pool.tile([P, 512], f32, tag="psu", name=f"psus{c}_{f}")
                        for k in range(KT):
                            nc.tensor.matmul(psg, shgu_sb[:, f, k, 0:P],
                                             xs[:, k * SCH : (k + 1) * SCH],
                                             start=(k == 0), stop=(k == KT - 1))
                            nc.tensor.matmul(psu, shgu_sb[:, f, k, P : 2 * P],
                                             xs[:, k * SCH : (k + 1) * SCH],
                                             start=(k == 0), stop=(k == KT - 1))
                        sil = tmp_pool.tile([P, 512], f32, tag="sil", name=f"sils{c}_{f}")
                        nc.scalar.activation(sil, psg, SILU)
                        nc.vector.tensor_mul(gush[:, f, :], sil, psu)
                    outsh = out_pool.tile([P, 16 * SCH], bf, tag="outsh", name=f"osh{c}")
                    for h in range(16):
                        psd = psd_pool.tile([P, 512], f32, tag="psd", name=f"psds{c}_{h}")
                        for kf in range(NFS):
                            nc.tensor.matmul(psd, shd_sb[:, kf, h * P : (h + 1) * P],
                                             gush[:, kf, :],
                                             start=(kf == 0), stop=(kf == NFS - 1))
                        nc.scalar.copy(outsh[:, h * SCH : (h + 1) * SCH], psd)
                    nc.gpsimd.dma_start(
                        ysh[:, c * 16 * SCH : (c + 1) * 16 * SCH], outsh)

    nc.compile()
    return nc


# ---------------------------------------------------------------------------
# Host-side routing / sharding / combine
# ---------------------------------------------------------------------------

def _route(x32, gate_w):
    logits = x32 @ gate_w.T.astype(np.float32)
    logits -= logits.max(-1, keepdims=True)
    np.exp(logits, out=logits)
    logits /= logits.sum(-1, keepdims=True)
    idx = np.argpartition(-logits, TOP_K - 1, axis=-1)[:, :TOP_K]
    w = np.take_along_axis(logits, idx, -1) * SCALE
    return idx.astype(np.int64), w.astype(np.float32)


def _align16(x):
    return min((int(x) + 15) // 16 * 16, 512)


def kernel(hidden_states, gate_w, w_gate, w_up, w_down, sh_gate, sh_up, sh_down):
    global LAST_EXEC_NS, _LAST_RUN, _LAST_CFG
    from concourse.bass_utils import run_bass_kernel_spmd

    x32 = np.ascontiguousarray(hidden_states, dtype=np.float32).reshape(T, H)

    # ---- host routing ----
    topk_idx, topk_w = _route(x32, np.asarray(gate_w, np.float32))
    eid = topk_idx.reshape(-1)
    order = np.argsort(eid, kind="stable")
    counts = np.bincount(eid, minlength=E)
    starts = np.concatenate([[0], np.cumsum(counts)[:-1]])
    wflat = topk_w.reshape(-1)

    # rank-group experts: slot j on all cores has width = max count in group j
    rank = np.argsort(-counts, kind="stable")          # experts by load desc
    groups = rank.reshape(8, N_CORES)                  # group j -> 8 experts
    gce = np.minimum(counts[groups], CAP)              # [8, 8] actual counts
    widths = tuple(_align16(gce[j].max()) for j in range(8))
    cfg = MoECfg(widths=widths)
    offs = cfg.offs

    # ---- per-core arrays ----
    x16T = np.ascontiguousarray(x32.T.astype(BF16))    # [H, T]
    wg16 = np.asarray(w_gate, np.float32).astype(BF16)
    wu16 = np.asarray(w_up, np.float32).astype(BF16)
    wdn16 = np.asarray(w_down, np.float32).astype(BF16)
    shg16 = np.asarray(sh_gate, np.float32).astype(BF16)
    shu16 = np.asarray(sh_up, np.float32).astype(BF16)
    shdn16 = np.asarray(sh_down, np.float32).astype(BF16)

    # xT: [NCH, P, KT*SCH] — per-chunk, per-partition contiguous
    xT_arr = np.ascontiguousarray(
        x16T.reshape(KT, P, NCH, SCH).transpose(2, 1, 0, 3).reshape(NCH, P, KT * SCH))

    def _gu_tiles(wmat):
        # [H, F] -> [NF, P, KT, P] lhsT tiles, partition-major for DMA
        return wmat.reshape(KT, P, NF, P).transpose(2, 1, 0, 3)

    in_maps = []
    core_meta = []
    for c in range(N_CORES):
        es = groups[:, c]                              # expert id per slot
        tok = np.zeros(cfg.S_pad, np.int64)
        meta = []
        for j in range(8):
            e = int(es[j])
            ce = int(min(counts[e], CAP))
            rows = order[starts[e] : starts[e] + ce]
            tok[offs[j] : offs[j] + ce] = rows // TOP_K
            meta.append((j, ce, rows))
        core_meta.append(meta)

        # xdT: [P, KT*S_pad], expert block j = [P, KT, wj] flattened at KT*off
        xdT_arr = np.empty((P, KT * cfg.S_pad), BF16)
        for j in range(8):
            wjj, offj = cfg.widths[j], offs[j]
            g = x16T[:, tok[offj : offj + wjj]]       # [H, wj]
            xdT_arr[:, KT * offj : KT * offj + KT * wjj] = (
                g.reshape(KT, P, wjj).transpose(1, 0, 2).reshape(P, KT * wjj))

        wgu_arr = np.empty((8, NF, P, KT, 2 * P), BF16)
        wd_arr = np.empty((8, 4, P, NF, 512), BF16)
        for j in range(8):
            e = int(es[j])
            wgu_arr[j, :, :, :, 0:P] = _gu_tiles(wg16[e])
            wgu_arr[j, :, :, :, P : 2 * P] = _gu_tiles(wu16[e])
            # [F, H] -> [NF, P, 4, 512] -> [4, P, NF, 512]
            wd_arr[j] = wdn16[e].reshape(NF, P, 4, 512).transpose(2, 1, 0, 3)

        sl = slice(c * FSH_LOC, (c + 1) * FSH_LOC)
        sgp = np.zeros((H, FSH_PAD), BF16)
        sup = np.zeros((H, FSH_PAD), BF16)
        sgp[:, :FSH_LOC] = shg16[sl].T
        sup[:, :FSH_LOC] = shu16[sl].T
        shgu_arr = np.empty((P, NFS, KT, 2 * P), BF16)
        shgu_arr[..., 0:P] = sgp.reshape(KT, P, NFS, P).transpose(1, 2, 0, 3)
        shgu_arr[..., P : 2 * P] = sup.reshape(KT, P, NFS, P).transpose(1, 2, 0, 3)
        sdp = np.zeros((FSH_PAD, H), BF16)
        sdp[:FSH_LOC] = shdn16[:, sl].T
        shd_arr = sdp.reshape(NFS, P, H).transpose(1, 0, 2)

        in_maps.append({
            "xdT": xdT_arr,
            "wgu": np.ascontiguousarray(wgu_arr),
            "wd": np.ascontiguousarray(wd_arr),
            "xT": xT_arr,
            "shgu": np.ascontiguousarray(shgu_arr),
            "shd": np.ascontiguousarray(shd_arr),
        })

    # ---- build + run ----
    nc = build_moe_program(cfg)
    trace = os.environ.get("MOE_TRACE", "0") == "1"
    res = run_bass_kernel_spmd(
        nc, in_maps, core_ids=list(range(N_CORES)), trace=trace,
    )
    LAST_EXEC_NS = res.exec_time_ns
    _LAST_RUN = (nc, in_maps)
    _LAST_CFG = cfg

    # ---- combine ----
    out_exp = np.zeros((T * TOP_K, H), np.float32)
    ysh_sum = np.zeros((P, 16 * T), np.float32)
    for c in range(N_CORES):
        dcore = np.asarray(res.results[c]["d_out"], dtype=np.float32)
        for j, ce, rows in core_meta[c]:
            wj, off = cfg.widths[j], offs[j]
            blk = dcore[:, 16 * off : 16 * off + 16 * wj].reshape(P, 16, wj)
            d_e = blk.transpose(2, 1, 0).reshape(wj, H)[:ce]
            out_exp[rows] = d_e * wflat[rows][:, None]
        ysh_sum += np.asarray(res.results[c]["ysh"], dtype=np.float32)
    y = out_exp.reshape(T, TOP_K, H).sum(axis=1)
    y += ysh_sum.reshape(P, NCH, 16, SCH).transpose(1, 3, 2, 0).reshape(T, H)
    return y.reshape(B, S, H).astype(hidden_states.dtype)


# revision 15
# speedup vs baseline: 1.7775x; 1.6582x over previous
"""DeepseekV2-Lite MoE (group GEMM) on 8 TRN2 NeuronCores — v2.

Strategy (expert-parallel, host-routed, weight-stationary ragged GEMMs):
  - Host (numpy, fp32): gate softmax -> top-6, per-expert token lists.
    Experts sorted by load, rank-grouped into 8 slots x 8 cores so every
    core runs an identical program with slot widths w_j = group max
    (aligned 16). Dispatch gathers x columns into [H, S_pad] per core.
  - Device (per core, Bass/Tile, bf16 MMs, fp32 PSUM):
      * routed: per expert slot j, weight-stationary SwiGLU:
          lhsT = weight tiles [128H x 128F], rhs = x columns [128H, w_j]
          -> g,u in PSUM [128F, w_j]; silu(g)*u -> gu SBUF (bf16);
          down: lhsT = w_down tiles [128F x 128H], rhs = gu -> out
          [128H, w_j]. Token dim rides the matmul free axis: no 128-row
          padding, no PE transposes. Gate weights applied on host.
      * shared experts: same flipped structure on an F_SH/8 slice over
        4 chunks of 512 tokens.
  - Host: unweighted slot outputs * gate weight, scatter-add; sum shared
    partials across cores.
"""

import os
from contextlib import ExitStack
from dataclasses import dataclass, field

import ml_dtypes
import numpy as np

H = 2048
F = 1408
E = 64
TOP_K = 6
CAP = 512
SCALE = 1.0
F_SH = 2 * F
B, S = 1, 2048
T = B * S
N_CORES = 8
P = 128
KT = H // P          # 16 k-tiles over H
NF = F // P          # 11 f-tiles over F
FSH_LOC = F_SH // N_CORES   # 352
FSH_PAD = 384
NFS = FSH_PAD // P   # 3
SCH = 256            # shared-phase token chunk (one per routed expert slot)
NCH = T // SCH       # 8 chunks

BF16 = ml_dtypes.bfloat16

LAST_EXEC_NS = None
_LAST_RUN = None
_LAST_CFG = None


@dataclass
class MoECfg:
    widths: tuple = (240,) * 8   # per-slot token widths (aligned 16, <=512)

    @property
    def offs(self):
        o, acc = [], 0
        for w in self.widths:
            o.append(acc)
            acc += w
        return o

    @property
    def S_pad(self):
        return sum(self.widths)

    @property
    def W_MAX(self):
        return max(self.widths)

    @property
    def OUTW(self):
        return 16 * self.S_pad


def build_moe_program(cfg: MoECfg, repeat: int = 1, no_wdma: bool = False):
    import contextlib
    import concourse.bass as bass  # noqa: F401
    import concourse.mybir as mybir
    from concourse import bacc
    import concourse.tile as tile

    bf = mybir.dt.bfloat16
    f32 = mybir.dt.float32
    SILU = mybir.ActivationFunctionType.Silu

    W_MAX, S_pad = cfg.W_MAX, cfg.S_pad

    nc = bacc.Bacc("TRN2", target_bir_lowering=False, debug=False)

    # ---- DRAM parameters (all pre-arranged host-side for fully-contiguous
    # per-partition DMA runs: no rearrange on the HBM side) -----------------
    xdT = nc.dram_tensor("xdT", [P, KT * S_pad], bf, kind="ExternalInput").ap()
    wgu = nc.dram_tensor("wgu", [8, NF, P, KT, 2 * P], bf, kind="ExternalInput").ap()
    wd = nc.dram_tensor("wd", [8, 4, P, NF, 512], bf, kind="ExternalInput").ap()
    xT = nc.dram_tensor("xT", [NCH, P, KT * SCH], bf, kind="ExternalInput").ap()
    shgu = nc.dram_tensor("shgu", [P, NFS, KT, 2 * P], bf, kind="ExternalInput").ap()
    shd = nc.dram_tensor("shd", [P, NFS, H], bf, kind="ExternalInput").ap()

    d_out = nc.dram_tensor("d_out", [P, cfg.OUTW], bf, kind="ExternalOutput").ap()
    ysh = nc.dram_tensor("ysh", [P, 16 * T], bf, kind="ExternalOutput").ap()

    with tile.TileContext(nc) as tc:
        with ExitStack() as top:
            shw_pool = top.enter_context(tc.tile_pool(name="shw", bufs=1))
            xd_pool = top.enter_context(tc.tile_pool(name="xd", bufs=2))
            wgu_pool = top.enter_context(tc.tile_pool(name="wgu", bufs=4))
            wd_pool = top.enter_context(tc.tile_pool(name="wd", bufs=2))
            gu_pool = top.enter_context(tc.tile_pool(name="gu", bufs=1))
            out_pool = top.enter_context(tc.tile_pool(name="out", bufs=2))
            tmp_pool = top.enter_context(tc.tile_pool(name="tmp", bufs=2))
            xsh_pool = top.enter_context(tc.tile_pool(name="xsh", bufs=2))
            gush_pool = top.enter_context(tc.tile_pool(name="gush", bufs=2))
            psgu_pool = top.enter_context(tc.tile_pool(name="psgu", bufs=2, space="PSUM"))
            psd_pool = top.enter_context(tc.tile_pool(name="psd", bufs=4, space="PSUM"))

            # shared-expert weights: prefetch once at start (scalar queue)
            shgu_sb = shw_pool.tile([P, NFS, KT, 2 * P], bf)
            nc.scalar.dma_start(shgu_sb, shgu)
            shd_sb = shw_pool.tile([P, NFS, H], bf)
            nc.scalar.dma_start(shd_sb, shd)

            # diagnostic mode: load one wgu/wd tile pair once, reuse for all
            # experts (garbage math, unchanged PE work, ~no weight DMA)
            if no_wdma:
                wt_fix = shw_pool.tile([P, KT, 2 * P], bf)
                nc.sync.dma_start(wt_fix, wgu[0, 0])
                wd_fix = shw_pool.tile([P, NF, 512], bf)
                nc.scalar.dma_start(wd_fix, wd[0, 0])

            WGU_Q = [nc.sync, nc.scalar, nc.gpsimd]
            rep_ctx = tc.For_i(0, repeat, 1) if repeat > 1 else contextlib.nullcontext()
            with rep_ctx:
                # routed expert j, then shared-expert token chunk j: interleaving
                # smooths DMA demand (routed needs ~17 MB/expert, shared ~1 MB)
                for j in range(8):
                    wj = cfg.widths[j]
                    off = cfg.offs[j]
                    xd = xd_pool.tile([P, KT * W_MAX], bf, tag="xd", name=f"xd{j}")
                    nc.sync.dma_start(
                        xd[:, : KT * wj],
                        xdT[:, KT * off : KT * off + KT * wj],
                    )
                    gu = gu_pool.tile([P, NF, W_MAX], bf, tag="gu", name=f"gu{j}")
                    for f in range(NF):
                        if no_wdma:
                            wt = wt_fix
                        else:
                            wt = wgu_pool.tile([P, KT, 2 * P], bf, tag="wgu",
                                               name=f"wgu{j}_{f}")
                            WGU_Q[f % 3].dma_start(wt, wgu[j, f])
                        psg = psgu_pool.tile([P, 512], f32, tag="psg", name=f"psg{j}_{f}")
                        psu = psgu_pool.tile([P, 512], f32, tag="psu", name=f"psu{j}_{f}")
                        for k in range(KT):
                            nc.tensor.matmul(psg[:, :wj], wt[:, k, 0:P],
                                             xd[:, k * wj : (k + 1) * wj],
                                             start=(k == 0), stop=(k == KT - 1))
                            nc.tensor.matmul(psu[:, :wj], wt[:, k, P : 2 * P],
                                             xd[:, k * wj : (k + 1) * wj],
                                             start=(k == 0), stop=(k == KT - 1))
                        sil = tmp_pool.tile([P, 512], f32, tag="sil", name=f"sil{j}_{f}")
                        nc.scalar.activation(sil[:, :wj], psg[:, :wj], SILU)
                        nc.vector.tensor_mul(gu[:, f, :wj], sil[:, :wj], psu[:, :wj])
                    outt = out_pool.tile([P, 16 * W_MAX], bf, tag="outt", name=f"out{j}")
                    for hc in range(4):  # H chunks of 512 cols
                        if no_wdma:
                            wdt = wd_fix
                        else:
                            wdt = wd_pool.tile([P, NF, 512], bf, tag="wd",
                                               name=f"wd{j}_{hc}")
                            (nc.scalar if hc % 2 == 0 else nc.gpsimd).dma_start(
                                wdt, wd[j, hc])
                        for h in range(4):
                            ht = hc * 4 + h
                            psd = psd_pool.tile([P, 512], f32, tag="psd",
                                                name=f"psd{j}_{ht}")
                            for kf in range(NF):
                                nc.tensor.matmul(
                                    psd[:, :wj], wdt[:, kf, h * P : (h + 1) * P],
                                    gu[:, kf, :wj],
                                    start=(kf == 0), stop=(kf == NF - 1),
                                )
                            nc.scalar.copy(outt[:, ht * wj : (ht + 1) * wj],
                                           psd[:, :wj])
                    nc.gpsimd.dma_start(
                        d_out[:, 16 * off : 16 * off + 16 * wj], outt[:, : 16 * wj])

                    # ---- shared-expert chunk j (F_SH/8 slice, 256 tokens) ----
                    c = j
                    xs = xsh_pool.tile([P, KT * SCH], bf, tag="xs", name=f"xs{c}")
                    nc.sync.dma_start(xs, xT[c])
                    gush = gush_pool.tile([P, NFS, SCH], bf, tag="gush", name=f"gush{c}")
                    for f in range(NFS):
                        psg = psgu_pool.tile([P, 512], f32, tag="psg", name=f"psgs{c}_{f}")
                        psu = psgu_pool.tile([P, 512], f32, tag="psu", name=f"psus{c}_{f}")
                        for k in range(KT):
                            nc.tensor.matmul(psg[:, :SCH], shgu_sb[:, f, k, 0:P],
                                             xs[:, k * SCH : (k + 1) * SCH],
                                             start=(k == 0), stop=(k == KT - 1))
                            nc.tensor.matmul(psu[:, :SCH], shgu_sb[:, f, k, P : 2 * P],
                                             xs[:, k * SCH : (k + 1) * SCH],
                                             start=(k == 0), stop=(k == KT - 1))
                        sil = tmp_pool.tile([P, 512], f32, tag="sil", name=f"sils{c}_{f}")
                        nc.scalar.activation(sil[:, :SCH], psg[:, :SCH], SILU)
                        nc.vector.tensor_mul(gush[:, f, :], sil[:, :SCH], psu[:, :SCH])
                    outsh = out_pool.tile([P, 16 * SCH], bf, tag="outsh", name=f"osh{c}")
                    for h in range(16):
                        psd = psd_pool.tile([P, 512], f32, tag="psd", name=f"psds{c}_{h}")
                        for kf in range(NFS):
                            nc.tensor.matmul(psd[:, :SCH],
                                             shd_sb[:, kf, h * P : (h + 1) * P],
                                             gush[:, kf, :],
                                             start=(kf == 0), stop=(kf == NFS - 1))
                        nc.scalar.copy(outsh[:, h * SCH : (h + 1) * SCH], psd[:, :SCH])
                    nc.scalar.dma_start(
                        ysh[:, c * 16 * SCH : (c + 1) * 16 * SCH], outsh)

    nc.compile()
    return nc


# ---------------------------------------------------------------------------
# Host-side routing / sharding / combine
# ---------------------------------------------------------------------------

def _route(x32, gate_w):
    logits = x32 @ gate_w.T.astype(np.float32)
    logits -= logits.max(-1, keepdims=True)
    np.exp(logits, out=logits)
    logits /= logits.sum(-1, keepdims=True)
    idx = np.argpartition(-logits, TOP_K - 1, axis=-1)[:, :TOP_K]
    w = np.take_along_axis(logits, idx, -1) * SCALE
    return idx.astype(np.int64), w.astype(np.float32)


def _align16(x):
    return min((int(x) + 15) // 16 * 16, 512)


def kernel(hidden_states, gate_w, w_gate, w_up, w_down, sh_gate, sh_up, sh_down):
    global LAST_EXEC_NS, _LAST_RUN, _LAST_CFG
    from concourse.bass_utils import run_bass_kernel_spmd

    x32 = np.ascontiguousarray(hidden_states, dtype=np.float32).reshape(T, H)

    # ---- host routing ----
    topk_idx, topk_w = _route(x32, np.asarray(gate_w, np.float32))
    eid = topk_idx.reshape(-1)
    order = np.argsort(eid, kind="stable")
    counts = np.bincount(eid, minlength=E)
    starts = np.concatenate([[0], np.cumsum(counts)[:-1]])
    wflat = topk_w.reshape(-1)

    # rank-group experts: slot j on all cores has width = max count in group j
    rank = np.argsort(-counts, kind="stable")          # experts by load desc
    groups = rank.reshape(8, N_CORES)                  # group j -> 8 experts
    gce = np.minimum(counts[groups], CAP)              # [8, 8] actual counts
    widths = tuple(_align16(gce[j].max()) for j in range(8))
    cfg = MoECfg(widths=widths)
    offs = cfg.offs

    # ---- per-core arrays ----
    x16T = np.ascontiguousarray(x32.T.astype(BF16))    # [H, T]
    wg16 = np.asarray(w_gate, np.float32).astype(BF16)
    wu16 = np.asarray(w_up, np.float32).astype(BF16)
    wdn16 = np.asarray(w_down, np.float32).astype(BF16)
    shg16 = np.asarray(sh_gate, np.float32).astype(BF16)
    shu16 = np.asarray(sh_up, np.float32).astype(BF16)
    shdn16 = np.asarray(sh_down, np.float32).astype(BF16)

    # xT: [NCH, P, KT*SCH] — per-chunk, per-partition contiguous
    xT_arr = np.ascontiguousarray(
        x16T.reshape(KT, P, NCH, SCH).transpose(2, 1, 0, 3).reshape(NCH, P, KT * SCH))

    def _gu_tiles(wmat):
        # [H, F] -> [NF, P, KT, P] lhsT tiles, partition-major for DMA
        return wmat.reshape(KT, P, NF, P).transpose(2, 1, 0, 3)

    in_maps = []
    core_meta = []
    for c in range(N_CORES):
        es = groups[:, c]                              # expert id per slot
        tok = np.zeros(cfg.S_pad, np.int64)
        meta = []
        for j in range(8):
            e = int(es[j])
            ce = int(min(counts[e], CAP))
            rows = order[starts[e] : starts[e] + ce]
            tok[offs[j] : offs[j] + ce] = rows // TOP_K
            meta.append((j, ce, rows))
        core_meta.append(meta)

        # xdT: [P, KT*S_pad], expert block j = [P, KT, wj] flattened at KT*off
        xdT_arr = np.empty((P, KT * cfg.S_pad), BF16)
        for j in range(8):
            wjj, offj = cfg.widths[j], offs[j]
            g = x16T[:, tok[offj : offj + wjj]]       # [H, wj]
            xdT_arr[:, KT * offj : KT * offj + KT * wjj] = (
                g.reshape(KT, P, wjj).transpose(1, 0, 2).reshape(P, KT * wjj))

        wgu_arr = np.empty((8, NF, P, KT, 2 * P), BF16)
        wd_arr = np.empty((8, 4, P, NF, 512), BF16)
        for j in range(8):
            e = int(es[j])
            wgu_arr[j, :, :, :, 0:P] = _gu_tiles(wg16[e])
            wgu_arr[j, :, :, :, P : 2 * P] = _gu_tiles(wu16[e])
            # [F, H] -> [NF, P, 4, 512] -> [4, P, NF, 512]
            wd_arr[j] = wdn16[e].reshape(NF, P, 4, 512).transpose(2, 1, 0, 3)

        sl = slice(c * FSH_LOC, (c + 1) * FSH_LOC)
        sgp = np.zeros((H, FSH_PAD), BF16)
        sup = np.zeros((H, FSH_PAD), BF16)
        sgp[:, :FSH_LOC] = shg16[sl].T
        sup[:, :FSH_LOC] = shu16[sl].T
        shgu_arr = np.empty((P, NFS, KT, 2 * P), BF16)
        shgu_arr[..., 0:P] = sgp.reshape(KT, P, NFS, P).transpose(1, 2, 0, 3)
        shgu_arr[..., P : 2 * P] = sup.reshape(KT, P, NFS, P).transpose(1, 2, 0, 3)
        sdp = np.zeros((FSH_PAD, H), BF16)
        sdp[:FSH_LOC] = shdn16[:, sl].T
        shd_arr = sdp.reshape(NFS, P, H).transpose(1, 0, 2)

        in_maps.append({
            "xdT": xdT_arr,
            "wgu": np.ascontiguousarray(wgu_arr),
            "wd": np.ascontiguousarray(wd_arr),
            "xT": xT_arr,
            "shgu": np.ascontiguousarray(shgu_arr),
            "shd": np.ascontiguousarray(shd_arr),
        })

    # ---- build + run ----
    nc = build_moe_program(cfg)
    trace = os.environ.get("MOE_TRACE", "0") == "1"
    res = run_bass_kernel_spmd(
        nc, in_maps, core_ids=list(range(N_CORES)), trace=trace,
    )
    LAST_EXEC_NS = res.exec_time_ns
    _LAST_RUN = (nc, in_maps)
    _LAST_CFG = cfg

    # ---- combine ----
    out_exp = np.zeros((T * TOP_K, H), np.float32)
    ysh_sum = np.zeros((P, 16 * T), np.float32)
    for c in range(N_CORES):
        dcore = np.asarray(res.results[c]["d_out"], dtype=np.float32)
        for j, ce, rows in core_meta[c]:
            wj, off = cfg.widths[j], offs[j]
            blk = dcore[:, 16 * off : 16 * off + 16 * wj].reshape(P, 16, wj)
            d_e = blk.transpose(2, 1, 0).reshape(wj, H)[:ce]
            out_exp[rows] = d_e * wflat[rows][:, None]
        ysh_sum += np.asarray(res.results[c]["ysh"], dtype=np.float32)
    y = out_exp.reshape(T, TOP_K, H).sum(axis=1)
    y += ysh_sum.reshape(P, NCH, 16, SCH).transpose(1, 3, 2, 0).reshape(T, H)
    return y.reshape(B, S, H).astype(hidden_states.dtype)
